# revision 1
# baseline (speedup 1.0000x reference)
"""Two-layer GCN encoder on 8 Trainium2 NeuronCores.

Strategy (dst-partitioned, matmul-based segment sum, fp16 internal):
  - Nodes are grouped into 392 blocks of 128; blocks are assigned to
    (core, slot) pairs balancing edge counts, 49 slots per core.
  - Every edge is owned by the core owning its dst block, so each core's
    aggregation for its blocks is complete: no all-reduce needed.
  - Node features live in "AllGather layout": row c*6272 + s*128 + off for
    the node at (core c, slot s, offset off).  Both layers gather from this
    layout with the SAME edge indices: layer 1 from x16_full (device-cast
    fp16 copy of x, assembled by an AllGather of per-core shards), layer 2
    from h_full (layer-1 activations, AllGather of fp16 shards).
  - Per edge tile (128 edges): dma_gather x rows into SBUF partitions,
    build P[e, n] = (iota == dstcol) * w with one fused DVE tensor_scalar
    (fp16 out), and accumulate aggT[feat, node] += Xg.T @ P in PSUM (fp32).
  - Per block: h = relu(aggT.T @ W + b) via two matmuls (bias as a K=1
    matmul) and an ACT relu eviction (fp16 for layer 1, fp32 output for
    layer 2).

dma_gather uses int16 indices (and hangs above ~1024 indices/call), so
gather sources are split at AG row 32768 (lo/hi) and calls are limited to
8 tiles.
"""

import numpy as np
from concourse import bacc, bass, mybir, tile
from concourse.bass_utils import run_bass_kernel_spmd

P = 128
N_NODES = 50000
N_EDGES = 800000
NFEAT = 128
NC = 8
SLOTS = 49                 # node blocks per core
NB = NC * SLOTS            # 392 blocks, 50176 padded rows
SHARD = SLOTS * P          # 6272 rows per core
NFULL = NB * P             # 50176
LO_SPLIT = 32768           # int16 index limit for dma_gather
GROUP = 5                  # slots per gather group
CALL_TILES = 8             # dma_gather hangs above ~1024 idxs/call

FP32 = mybir.dt.float32
FP16 = mybir.dt.float16

# Set by kernel() for test harness introspection (trace results etc.)
last_run_results = None


def _wrap16(flat):
    """dma_gather index layout: logical i -> [i % 16, i // 16], x8 replicated."""
    n16 = len(flat) // 16
    arr = np.asarray(flat, dtype=np.int16).reshape(n16, 16).T  # [16, n16]
    return np.tile(arr, (8, 1))  # [128, n16]


def _prep(edge_index, edge_weight):
    """Host-side sharding: block assignment, gather indices (AG layout), colw."""
    src = edge_index[0].astype(np.int64)
    dst = edge_index[1].astype(np.int64)
    w = edge_weight.astype(np.float32)

    blk = dst >> 7
    col = (dst & 127).astype(np.float32)

    cnt = np.bincount(blk, minlength=NB)
    order = np.argsort(-cnt, kind="stable")
    # Refine within slabs of 4 slots: re-sort by lo-edge count so each
    # slot's 8 blocks have similar lo/hi splits (reduces the shared
    # max-over-cores tile schedule).  The AG-row threshold depends on the
    # assignment itself, so approximate lo-ness with a first-pass
    # assignment by total count.
    core_of0 = np.empty(NB, np.int64)
    slot_of0 = np.empty(NB, np.int64)
    ba0 = order.reshape(SLOTS, NC).T
    for c0 in range(NC):
        for s0 in range(SLOTS):
            core_of0[ba0[c0, s0]] = c0
            slot_of0[ba0[c0, s0]] = s0
    sblk0 = src >> 7
    v0 = core_of0[sblk0] * SHARD + slot_of0[sblk0] * P + (src & 127)
    lo_cnt = np.bincount(blk[v0 < LO_SPLIT], minlength=NB)
    order2 = order.copy()
    for a in range(0, NB, 4 * NC):
        slab = order2[a:a + 4 * NC]
        order2[a:a + 4 * NC] = slab[np.argsort(-lo_cnt[slab], kind="stable")]
    block_at = order2.reshape(SLOTS, NC).T          # [core, slot] -> block
    core_of = np.empty(NB, np.int64)
    slot_of = np.empty(NB, np.int64)
    for c in range(NC):
        for s in range(SLOTS):
            core_of[block_at[c, s]] = c
            slot_of[block_at[c, s]] = s

    eorder = np.argsort(blk, kind="stable")
    estart = np.zeros(NB + 1, np.int64)
    np.cumsum(cnt, out=estart[1:])

    # gather index (AllGather-layout row) for each edge's src
    sblk = src >> 7
    v = core_of[sblk] * SHARD + slot_of[sblk] * P + (src & 127)

    groups = [list(range(g, min(g + GROUP, SLOTS))) for g in range(0, SLOTS, GROUP)]

    # per (core, slot): lo/hi edge id lists + shared tile schedule
    ids_cs = [[None] * SLOTS for _ in range(NC)]
    LT = np.zeros(SLOTS, np.int64)
    HT = np.zeros(SLOTS, np.int64)
    for c in range(NC):
        for s in range(SLOTS):
            b = block_at[c, s]
            ids = eorder[estart[b]:estart[b + 1]]
            m = v[ids] < LO_SPLIT
            lo, hi = ids[m], ids[~m]
            ids_cs[c][s] = (lo, hi)
            LT[s] = max(LT[s], (len(lo) + P - 1) // P)
            HT[s] = max(HT[s], (len(hi) + P - 1) // P)

    # Tile enumeration: for g in groups: for part in (lo, hi): for s in g.
    gdescs = []
    tid0 = 0
    for g in groups:
        lo_tiles = int(sum(LT[s] for s in g))
        hi_tiles = int(sum(HT[s] for s in g))
        gdescs.append({
            "slots": g, "lo_tiles": lo_tiles, "hi_tiles": hi_tiles, "tid0": tid0,
        })
        tid0 += lo_tiles + hi_tiles
    sched = {"LT": LT, "HT": HT, "groups": gdescs, "ntiles": tid0}

    # Gather calls: one per (group, part, slot, <=CALL_TILES window).  Idx
    # streams are padded with -1 (the Q7 ucode trims trailing negatives, so
    # padding costs no descriptors); num_idxs_reg is the max real count over
    # cores (shared SPMD immediate, only used for ring-space reservation).
    nreal = np.zeros((NC, SLOTS, 2), np.int64)
    for c in range(NC):
        for s in range(SLOTS):
            lo, hi = ids_cs[c][s]
            nreal[c, s, 0] = len(lo)
            nreal[c, s, 1] = len(hi)
    calls = []  # (group, part, tile0_in_group_part, ntiles)
    for gi, g in enumerate(groups):
        for part in range(2):
            T = LT if part == 0 else HT
            ptiles = int(sum(T[s] for s in g))
            t0 = 0
            while t0 < ptiles:
                nt = min(CALL_TILES, ptiles - t0)
                calls.append((gi, part, t0, nt))
                t0 += nt
    sched["calls"] = calls
    counts_np = np.zeros((NC, 1), np.int32)  # unused placeholder

    idx_np = []
    colw_np = []
    for c in range(NC):
        flat_idx = []
        colw = np.zeros((P, 2 * tid0), np.float32)
        tid = 0
        for g in groups:
            for part in range(2):
                T = LT if part == 0 else HT
                for s in g:
                    lo, hi = ids_cs[c][s]
                    ids = lo if part == 0 else hi
                    n = int(T[s]) * P
                    iv = np.zeros(n, np.int64)
                    cv = np.zeros(n, np.float32)
                    wv = np.zeros(n, np.float32)
                    iv[:len(ids)] = v[ids] - (0 if part == 0 else LO_SPLIT)
                    cv[:len(ids)] = col[ids]
                    wv[:len(ids)] = w[ids]
                    flat_idx.append(iv)
                    for t in range(int(T[s])):
                        colw[:, 2 * tid] = cv[t * P:(t + 1) * P]
                        colw[:, 2 * tid + 1] = wv[t * P:(t + 1) * P]
                        tid += 1
        idx_np.append(_wrap16(np.concatenate(flat_idx)))
        colw_np.append(colw)

    return block_at, sched, idx_np, colw_np, counts_np


def _build(sched, n16):
    """Build the SPMD bass program. Returns finalized nc."""
    nc = bacc.Bacc(num_devices=NC)

    xsh_in = nc.declare_dram_parameter("xsh", [SHARD, NFEAT], FP32, isOutput=False)
    w1_in = nc.declare_dram_parameter("W1", [NFEAT, NFEAT], FP32, isOutput=False)
    w2_in = nc.declare_dram_parameter("W2", [NFEAT, NFEAT], FP32, isOutput=False)
    b1_in = nc.declare_dram_parameter("b1", [1, NFEAT], FP32, isOutput=False)
    b2_in = nc.declare_dram_parameter("b2", [1, NFEAT], FP32, isOutput=False)
    iota_in = nc.declare_dram_parameter("iota", [P, P], FP32, isOutput=False)
    idx_in = nc.declare_dram_parameter("idx", [P, n16], mybir.dt.int16, isOutput=False)
    colw_in = nc.declare_dram_parameter(
        "colw", [P, 2 * sched["ntiles"]], FP32, isOutput=False
    )
    out = nc.declare_dram_parameter("out", [SHARD, NFEAT], FP32, isOutput=True)

    relu = mybir.ActivationFunctionType.Relu
    XCONV = 7  # blocks converted per fp16-cast chunk (49 = 7*7)

    with tile.TileContext(nc) as tc:
        with tc.tile_pool(name="const", bufs=1) as cpool, \
             tc.tile_pool(name="xc", bufs=2) as xcpool, \
             tc.tile_pool(name="gbuf", bufs=3) as gpool, \
             tc.tile_pool(name="pmat", bufs=16) as ppool, \
             tc.tile_pool(name="evict", bufs=3) as epool, \
             tc.tile_pool(name="hout", bufs=3) as hpool, \
             tc.tile_pool(name="psA", bufs=4, space="PSUM") as psA, \
             tc.tile_pool(name="psB", bufs=2, space="PSUM") as psB, \
             tc.tile_pool(name="dram", bufs=1, space="DRAM") as dpool:

            iota_t = cpool.tile([P, P], FP32)
            w_t = [cpool.tile([P, P], FP16, name=f"w{l}") for l in range(2)]
            b_t = [cpool.tile([1, P], FP16, name=f"b{l}") for l in range(2)]
            wld_t = [cpool.tile([P, P], FP32, name=f"wld{l}") for l in range(2)]
            bld_t = [cpool.tile([1, P], FP32, name=f"bld{l}") for l in range(2)]
            ones_t = cpool.tile([1, P], FP16)
            idx_t = cpool.tile([P, n16], mybir.dt.int16)
            colw_t = cpool.tile([P, 2 * sched["ntiles"]], FP32)

            nc.sync.dma_start(out=iota_t[:], in_=iota_in[:])
            for l, (wi, bi) in enumerate([(w1_in, b1_in), (w2_in, b2_in)]):
                nc.sync.dma_start(out=wld_t[l][:], in_=wi[:])
                nc.sync.dma_start(out=bld_t[l][:], in_=bi[:])
                nc.vector.tensor_copy(out=w_t[l][:], in_=wld_t[l][:])
                nc.vector.tensor_copy(out=b_t[l][:], in_=bld_t[l][:])
            nc.vector.memset(ones_t[:], 1.0)
            nc.sync.dma_start(out=idx_t[:], in_=idx_in[:])
            nc.sync.dma_start(out=colw_t[:], in_=colw_in[:])

            x16_shard = dpool.tile([SHARD, NFEAT], FP16, name="x16_shard")
            x16_full = dpool.tile([NFULL, NFEAT], FP16, name="x16_full")
            h_shard = dpool.tile([SHARD, NFEAT], FP16, name="h_shard")
            h_full = dpool.tile([NFULL, NFEAT], FP16, name="h_full")

            # fp32 -> fp16 cast of this core's x shard, then AllGather.
            # Batched XCONV blocks per chunk via manual 3D APs
            # ([partition, block, feat]; strides in elements).
            xsh_ap = xsh_in[:]
            x16sh_ap = x16_shard[:]
            for j in range(0, SLOTS, XCONV):
                n = min(XCONV, SLOTS - j)
                xc32 = xcpool.tile([P, XCONV * NFEAT], FP32, name="xc32", tag="xc32")
                xc16 = xcpool.tile([P, XCONV * NFEAT], FP16, name="xc16", tag="xc16")
                src3 = bass.AP(
                    xsh_ap.tensor, j * P * NFEAT,
                    [[NFEAT, P], [P * NFEAT, n], [1, NFEAT]],
                )
                dst3 = bass.AP(
                    x16sh_ap.tensor, j * P * NFEAT,
                    [[NFEAT, P], [P * NFEAT, n], [1, NFEAT]],
                )
                nc.sync.dma_start(out=xc32[:, :n * NFEAT], in_=src3)
                nc.vector.tensor_copy(out=xc16[:, :n * NFEAT], in_=xc32[:, :n * NFEAT])
                nc.sync.dma_start(out=dst3, in_=xc16[:, :n * NFEAT])
            nc.gpsimd.collective_compute(
                "AllGather", mybir.AluOpType.bypass,
                replica_groups=[list(range(NC))],
                ins=[x16_shard[:]], outs=[x16_full[:]],
            )

            def layer(l, src_lo, src_hi, dst_ap, out_dt):
                LT, HT = sched["LT"], sched["HT"]
                for gi, gd in enumerate(sched["groups"]):
                    slots = gd["slots"]
                    lo_tiles, hi_tiles = gd["lo_tiles"], gd["hi_tiles"]
                    all_tiles = lo_tiles + hi_tiles
                    gbuf = gpool.tile([P, all_tiles * P], FP16, name="gbuf", tag="gbuf")
                    for cgi, cpart, ct0, cnt in sched["calls"]:
                        if cgi != gi:
                            continue
                        pos = (0 if cpart == 0 else lo_tiles) + ct0
                        gtid = gd["tid0"] + pos
                        nidx = cnt * P
                        srcap = src_lo if cpart == 0 else src_hi
                        nc.gpsimd.dma_gather(
                            out_ap=gbuf[:, pos * P:pos * P + nidx].rearrange(
                                "p (t e) -> p t e", e=P
                            ),
                            in_ap=srcap,
                            idxs_ap=idx_t[:, gtid * 8:gtid * 8 + nidx // 16],
                            num_idxs=nidx,
                            num_idxs_reg=nidx,
                            elem_size=P,
                        )
                    # per-slot tile ranges within gbuf; colw tile id for gbuf
                    # tile gt is tid0 + gt (same (part, slot) enumeration)
                    lo_base = 0
                    hi_base = lo_tiles
                    for s in slots:
                        nlo, nhi = int(LT[s]), int(HT[s])
                        tlist = [lo_base + t for t in range(nlo)] + \
                                [hi_base + t for t in range(nhi)]
                        lo_base += nlo
                        hi_base += nhi
                        ntot = nlo + nhi
                        aggT = psA.tile([P, P], FP32, space="PSUM", name="aggT", tag="aggT")
                        for k, gt in enumerate(tlist):
                            tid = gd["tid0"] + gt
                            pm = ppool.tile([P, P], FP16, name="pm", tag="pm")
                            nc.vector.tensor_scalar(
                                out=pm[:],
                                in0=iota_t[:],
                                scalar1=colw_t[:, 2 * tid:2 * tid + 1],
                                scalar2=colw_t[:, 2 * tid + 1:2 * tid + 2],
                                op0=mybir.AluOpType.is_equal,
                                op1=mybir.AluOpType.mult,
                            )
                            nc.tensor.matmul(
                                out=aggT[:],
                                lhsT=gbuf[:, gt * P:(gt + 1) * P],
                                rhs=pm[:],
                                start=(k == 0),
                                stop=(k == ntot - 1),
                            )
                        aggT_sb = epool.tile([P, P], FP16, name="evict", tag="evict")
                        nc.scalar.copy(out=aggT_sb[:], in_=aggT[:])
                        h_ps = psB.tile([P, P], FP32, space="PSUM", name="hps", tag="hps")
                        nc.tensor.matmul(
                            out=h_ps[:], lhsT=aggT_sb[:], rhs=w_t[l][:],
                            start=True, stop=False,
                        )
                        nc.tensor.matmul(
                            out=h_ps[:], lhsT=ones_t[0:1, :], rhs=b_t[l][0:1, :],
                            start=False, stop=True,
                        )
                        h_sb = hpool.tile([P, P], out_dt, name="hout", tag=f"hout{l}")
                        nc.scalar.activation(out=h_sb[:], in_=h_ps[:], func=relu)
                        nc.sync.dma_start(
                            out=dst_ap[s * P:(s + 1) * P, :], in_=h_sb[:]
                        )

            layer(0, x16_full[0:LO_SPLIT, :], x16_full[LO_SPLIT:NFULL, :],
                  h_shard[:], FP16)

            nc.gpsimd.collective_compute(
                "AllGather", mybir.AluOpType.bypass,
                replica_groups=[list(range(NC))],
                ins=[h_shard[:]], outs=[h_full[:]],
            )

            layer(1, h_full[0:LO_SPLIT, :], h_full[LO_SPLIT:NFULL, :],
                  out[:], FP32)

    nc.finalize()
    return nc


def kernel(x, edge_index, edge_weight, W1, b1, W2, b2):
    global last_run_results
    x = np.ascontiguousarray(np.asarray(x, dtype=np.float32))
    edge_index = np.asarray(edge_index)
    edge_weight = np.asarray(edge_weight, dtype=np.float32)

    block_at, sched, idx_np, colw_np, counts_np = _prep(edge_index, edge_weight)
    n16 = idx_np[0].shape[1]
    nc = _build(sched, n16)

    iota_np = np.broadcast_to(np.arange(P, dtype=np.float32), (P, P)).copy()
    xpad = np.zeros((NFULL, NFEAT), np.float32)
    xpad[:N_NODES] = x
    in_maps = []
    for c in range(NC):
        xsh = np.concatenate(
            [xpad[b * P:(b + 1) * P] for b in block_at[c]], axis=0
        )
        in_maps.append({
            "xsh": np.ascontiguousarray(xsh),
            "W1": np.ascontiguousarray(W1, dtype=np.float32),
            "W2": np.ascontiguousarray(W2, dtype=np.float32),
            "b1": np.ascontiguousarray(b1, dtype=np.float32).reshape(1, NFEAT),
            "b2": np.ascontiguousarray(b2, dtype=np.float32).reshape(1, NFEAT),
            "iota": iota_np,
            "idx": idx_np[c],
            "colw": colw_np[c],
        })

    import os
    trace = bool(int(os.environ.get("GCN_TRACE", "0")))
    res = run_bass_kernel_spmd(nc, in_maps, list(range(NC)), trace=trace)
    last_run_results = res

    full = np.zeros((NFULL, NFEAT), np.float32)
    for c in range(NC):
        shard = res.results[c]["out"]
        for s in range(SLOTS):
            b = int(block_at[c, s])
            full[b * P:(b + 1) * P] = shard[s * P:(s + 1) * P]
    return full[:N_NODES]



# revision 8
# speedup vs baseline: 1.7553x; 1.7553x over previous
"""Two-layer GCN encoder on 8 Trainium2 NeuronCores.

Strategy (dst-partitioned, matmul-based segment sum, fp16 internal):
  - Nodes are grouped into 392 blocks of 128; blocks are assigned to
    (core, slot) pairs balancing edge counts, 49 slots per core.
  - Every edge is owned by the core owning its dst block, so each core's
    aggregation for its blocks is complete: no all-reduce needed.
  - Node features live in "AllGather layout": row c*6272 + s*128 + off for
    the node at (core c, slot s, offset off).  Both layers gather from this
    layout with the SAME edge indices: layer 1 from x16_full (device-cast
    fp16 copy of x, assembled by an AllGather of per-core shards), layer 2
    from h_full (layer-1 activations, AllGather of fp16 shards).
  - Per edge tile (128 edges): dma_gather x rows into SBUF partitions,
    build P[e, n] = (iota == dstcol) * w with one fused DVE tensor_scalar
    (fp16 out), and accumulate aggT[feat, node] += Xg.T @ P in PSUM (fp32).
  - Per block: h = relu(aggT.T @ W + b) via two matmuls (bias as a K=1
    matmul) and an ACT relu eviction (fp16 for layer 1, fp32 output for
    layer 2).

dma_gather uses int16 indices (and hangs above ~1024 indices/call), so
gather sources are split at AG row 32768 (lo/hi) and calls are limited to
8 tiles.
"""

import numpy as np
from concourse import bacc, bass, mybir, tile
from concourse.bass_utils import run_bass_kernel_spmd

P = 128
N_NODES = 50000
N_EDGES = 800000
NFEAT = 128
NC = 8
SLOTS = 49                 # node blocks per core
NB = NC * SLOTS            # 392 blocks, 50176 padded rows
SHARD = SLOTS * P          # 6272 rows per core
NFULL = NB * P             # 50176
LO_SPLIT = 32768           # int16 index limit for dma_gather
GROUP = 5                  # slots per gather group
CALL_TILES = 8             # dma_gather hangs above ~1024 idxs/call

FP32 = mybir.dt.float32
FP16 = mybir.dt.float16

# Set by kernel() for test harness introspection (trace results etc.)
last_run_results = None


def _wrap16(flat):
    """dma_gather index layout: logical i -> [i % 16, i // 16], x8 replicated."""
    n16 = len(flat) // 16
    arr = np.asarray(flat, dtype=np.int16).reshape(n16, 16).T  # [16, n16]
    return np.tile(arr, (8, 1))  # [128, n16]


def _prep(edge_index, edge_weight):
    """Host-side sharding: block assignment, gather indices (AG layout), colw."""
    src = edge_index[0].astype(np.int64)
    dst = edge_index[1].astype(np.int64)
    w = edge_weight.astype(np.float32)

    blk = dst >> 7
    col = (dst & 127).astype(np.float32)

    cnt = np.bincount(blk, minlength=NB)
    order = np.argsort(-cnt, kind="stable")
    # Refine within slabs of 4 slots: re-sort by lo-edge count so each
    # slot's 8 blocks have similar lo/hi splits (reduces the shared
    # max-over-cores tile schedule).  The AG-row threshold depends on the
    # assignment itself, so approximate lo-ness with a first-pass
    # assignment by total count.
    core_of0 = np.empty(NB, np.int64)
    slot_of0 = np.empty(NB, np.int64)
    ba0 = order.reshape(SLOTS, NC).T
    for c0 in range(NC):
        for s0 in range(SLOTS):
            core_of0[ba0[c0, s0]] = c0
            slot_of0[ba0[c0, s0]] = s0
    sblk0 = src >> 7
    v0 = core_of0[sblk0] * SHARD + slot_of0[sblk0] * P + (src & 127)
    lo_cnt = np.bincount(blk[v0 < LO_SPLIT], minlength=NB)
    order2 = order.copy()
    for a in range(0, NB, 4 * NC):
        slab = order2[a:a + 4 * NC]
        order2[a:a + 4 * NC] = slab[np.argsort(-lo_cnt[slab], kind="stable")]
    block_at = order2.reshape(SLOTS, NC).T          # [core, slot] -> block
    core_of = np.empty(NB, np.int64)
    slot_of = np.empty(NB, np.int64)
    for c in range(NC):
        for s in range(SLOTS):
            core_of[block_at[c, s]] = c
            slot_of[block_at[c, s]] = s

    eorder = np.argsort(blk, kind="stable")
    estart = np.zeros(NB + 1, np.int64)
    np.cumsum(cnt, out=estart[1:])

    # gather index (AllGather-layout row) for each edge's src
    sblk = src >> 7
    v = core_of[sblk] * SHARD + slot_of[sblk] * P + (src & 127)

    groups = [list(range(g, min(g + GROUP, SLOTS))) for g in range(0, SLOTS, GROUP)]

    # per (core, slot): lo/hi edge id lists + shared tile schedule
    ids_cs = [[None] * SLOTS for _ in range(NC)]
    LT = np.zeros(SLOTS, np.int64)
    HT = np.zeros(SLOTS, np.int64)
    for c in range(NC):
        for s in range(SLOTS):
            b = block_at[c, s]
            ids = eorder[estart[b]:estart[b + 1]]
            m = v[ids] < LO_SPLIT
            lo, hi = ids[m], ids[~m]
            ids_cs[c][s] = (lo, hi)
            LT[s] = max(LT[s], (len(lo) + P - 1) // P)
            HT[s] = max(HT[s], (len(hi) + P - 1) // P)

    # Tile enumeration: for g in groups: for part in (lo, hi): for s in g.
    gdescs = []
    tid0 = 0
    for g in groups:
        lo_tiles = int(sum(LT[s] for s in g))
        hi_tiles = int(sum(HT[s] for s in g))
        gdescs.append({
            "slots": g, "lo_tiles": lo_tiles, "hi_tiles": hi_tiles, "tid0": tid0,
        })
        tid0 += lo_tiles + hi_tiles
    sched = {"LT": LT, "HT": HT, "groups": gdescs, "ntiles": tid0}

    # Gather calls: one per (group, part, slot, <=CALL_TILES window).  Idx
    # streams are padded with -1 (the Q7 ucode trims trailing negatives, so
    # padding costs no descriptors); num_idxs_reg is the max real count over
    # cores (shared SPMD immediate, only used for ring-space reservation).
    nreal = np.zeros((NC, SLOTS, 2), np.int64)
    for c in range(NC):
        for s in range(SLOTS):
            lo, hi = ids_cs[c][s]
            nreal[c, s, 0] = len(lo)
            nreal[c, s, 1] = len(hi)
    calls = []  # (group, part, tile0_in_group_part, ntiles)
    for gi, g in enumerate(groups):
        for part in range(2):
            T = LT if part == 0 else HT
            ptiles = int(sum(T[s] for s in g))
            t0 = 0
            while t0 < ptiles:
                nt = min(CALL_TILES, ptiles - t0)
                calls.append((gi, part, t0, nt))
                t0 += nt
    sched["calls"] = calls
    counts_np = np.zeros((NC, 1), np.int32)  # unused placeholder

    idx_np = []
    colw_np = []
    for c in range(NC):
        flat_idx = []
        colw = np.zeros((P, 2 * tid0), np.float32)
        tid = 0
        for g in groups:
            for part in range(2):
                T = LT if part == 0 else HT
                for s in g:
                    lo, hi = ids_cs[c][s]
                    ids = lo if part == 0 else hi
                    n = int(T[s]) * P
                    iv = np.zeros(n, np.int64)
                    cv = np.zeros(n, np.float32)
                    wv = np.zeros(n, np.float32)
                    iv[:len(ids)] = v[ids] - (0 if part == 0 else LO_SPLIT)
                    cv[:len(ids)] = col[ids]
                    wv[:len(ids)] = w[ids]
                    flat_idx.append(iv)
                    for t in range(int(T[s])):
                        colw[:, 2 * tid] = cv[t * P:(t + 1) * P]
                        colw[:, 2 * tid + 1] = wv[t * P:(t + 1) * P]
                        tid += 1
        idx_np.append(_wrap16(np.concatenate(flat_idx)))
        colw_np.append(colw)

    return block_at, sched, idx_np, colw_np, counts_np


def _build(sched, n16):
    """Build the SPMD bass program. Returns finalized nc."""
    nc = bacc.Bacc(num_devices=NC, num_swdge_queues=4)

    xsh_in = nc.declare_dram_parameter("xsh", [SHARD, NFEAT], FP32, isOutput=False)
    w1_in = nc.declare_dram_parameter("W1", [NFEAT, NFEAT], FP32, isOutput=False)
    w2_in = nc.declare_dram_parameter("W2", [NFEAT, NFEAT], FP32, isOutput=False)
    b1_in = nc.declare_dram_parameter("b1", [1, NFEAT], FP32, isOutput=False)
    b2_in = nc.declare_dram_parameter("b2", [1, NFEAT], FP32, isOutput=False)
    iota_in = nc.declare_dram_parameter("iota", [P, P], FP32, isOutput=False)
    idx_in = nc.declare_dram_parameter("idx", [P, n16], mybir.dt.int16, isOutput=False)
    colw_in = nc.declare_dram_parameter(
        "colw", [P, 2 * sched["ntiles"]], FP32, isOutput=False
    )
    out = nc.declare_dram_parameter("out", [SHARD, NFEAT], FP32, isOutput=True)

    relu = mybir.ActivationFunctionType.Relu
    XCONV = 7  # blocks converted per fp16-cast chunk (49 = 7*7)

    with tile.TileContext(nc) as tc:
        with tc.tile_pool(name="const", bufs=1) as cpool, \
             tc.tile_pool(name="xc", bufs=2) as xcpool, \
             tc.tile_pool(name="gbuf", bufs=3) as gpool, \
             tc.tile_pool(name="pmat", bufs=16) as ppool, \
             tc.tile_pool(name="evict", bufs=3) as epool, \
             tc.tile_pool(name="hout", bufs=3) as hpool, \
             tc.tile_pool(name="psA", bufs=4, space="PSUM") as psA, \
             tc.tile_pool(name="psB", bufs=2, space="PSUM") as psB, \
             tc.tile_pool(name="psC", bufs=1, space="PSUM") as psC, \
             tc.tile_pool(name="dram", bufs=1, space="DRAM") as dpool:

            iota_t = cpool.tile([P, P], FP32)
            w_t = [cpool.tile([P, P], FP16, name=f"w{l}") for l in range(2)]
            b_t = [cpool.tile([1, P], FP16, name=f"b{l}") for l in range(2)]
            wld_t = [cpool.tile([P, P], FP32, name=f"wld{l}") for l in range(2)]
            bld_t = [cpool.tile([1, P], FP32, name=f"bld{l}") for l in range(2)]
            ones_t = cpool.tile([1, P], FP16)
            idx_t = cpool.tile([P, n16], mybir.dt.int16)
            colw_t = cpool.tile([P, 2 * sched["ntiles"]], FP32)

            # iota lives in PSUM so the per-tile P-build tensor_scalar runs in
            # 1x mode (PSUM src): it then never takes the SBUF port pair that
            # GpSimd needs for SWDGE descriptor writes, so DVE ops and
            # dma_gather descriptor generation don't serialize each other.
            iota_ps = psC.tile([P, P], FP32, space="PSUM", name="iotaps")
            nc.sync.dma_start(out=iota_t[:], in_=iota_in[:])
            nc.vector.tensor_copy(out=iota_ps[:], in_=iota_t[:])
            for l, (wi, bi) in enumerate([(w1_in, b1_in), (w2_in, b2_in)]):
                nc.sync.dma_start(out=wld_t[l][:], in_=wi[:])
                nc.sync.dma_start(out=bld_t[l][:], in_=bi[:])
                nc.scalar.copy(out=w_t[l][:], in_=wld_t[l][:])
                nc.scalar.copy(out=b_t[l][:], in_=bld_t[l][:])
            nc.vector.memset(ones_t[:], 1.0)
            nc.sync.dma_start(out=idx_t[:], in_=idx_in[:])
            nc.sync.dma_start(out=colw_t[:], in_=colw_in[:])

            x16_shard = dpool.tile([SHARD, NFEAT], FP16, name="x16_shard")
            x16_full = dpool.tile([NFULL, NFEAT], FP16, name="x16_full")
            h_shard = dpool.tile([SHARD, NFEAT], FP16, name="h_shard")
            h_full = dpool.tile([NFULL, NFEAT], FP16, name="h_full")

            # fp32 -> fp16 cast of this core's x shard, then AllGather.
            # Batched XCONV blocks per chunk via manual 3D APs
            # ([partition, block, feat]; strides in elements).
            xsh_ap = xsh_in[:]
            x16sh_ap = x16_shard[:]
            for j in range(0, SLOTS, XCONV):
                n = min(XCONV, SLOTS - j)
                xc32 = xcpool.tile([P, XCONV * NFEAT], FP32, name="xc32", tag="xc32")
                xc16 = xcpool.tile([P, XCONV * NFEAT], FP16, name="xc16", tag="xc16")
                src3 = bass.AP(
                    xsh_ap.tensor, j * P * NFEAT,
                    [[NFEAT, P], [P * NFEAT, n], [1, NFEAT]],
                )
                dst3 = bass.AP(
                    x16sh_ap.tensor, j * P * NFEAT,
                    [[NFEAT, P], [P * NFEAT, n], [1, NFEAT]],
                )
                nc.sync.dma_start(out=xc32[:, :n * NFEAT], in_=src3)
                nc.scalar.copy(out=xc16[:, :n * NFEAT], in_=xc32[:, :n * NFEAT])
                nc.sync.dma_start(out=dst3, in_=xc16[:, :n * NFEAT])
            nc.gpsimd.collective_compute(
                "AllGather", mybir.AluOpType.bypass,
                replica_groups=[list(range(NC))],
                ins=[x16_shard[:]], outs=[x16_full[:]],
            )

            call_q = [0]  # round-robin SWDGE queue so descriptor generation
                          # spreads across all four Q7 core pairs

            def layer(l, src_lo, src_hi, dst_ap, out_dt):
                LT, HT = sched["LT"], sched["HT"]
                for gi, gd in enumerate(sched["groups"]):
                    slots = gd["slots"]
                    lo_tiles, hi_tiles = gd["lo_tiles"], gd["hi_tiles"]
                    all_tiles = lo_tiles + hi_tiles
                    gbuf = gpool.tile([P, all_tiles * P], FP16, name="gbuf", tag="gbuf")
                    for cgi, cpart, ct0, cnt in sched["calls"]:
                        if cgi != gi:
                            continue
                        pos = (0 if cpart == 0 else lo_tiles) + ct0
                        gtid = gd["tid0"] + pos
                        nidx = cnt * P
                        srcap = src_lo if cpart == 0 else src_hi
                        nc.gpsimd.dma_gather(
                            out_ap=gbuf[:, pos * P:pos * P + nidx].rearrange(
                                "p (t e) -> p t e", e=P
                            ),
                            in_ap=srcap,
                            idxs_ap=idx_t[:, gtid * 8:gtid * 8 + nidx // 16],
                            num_idxs=nidx,
                            num_idxs_reg=nidx,
                            elem_size=P,
                            queue_num=call_q[0] % 4,
                        )
                        call_q[0] += 1
                    # per-slot tile ranges within gbuf; colw tile id for gbuf
                    # tile gt is tid0 + gt (same (part, slot) enumeration)
                    lo_base = 0
                    hi_base = lo_tiles
                    for s in slots:
                        nlo, nhi = int(LT[s]), int(HT[s])
                        tlist = [lo_base + t for t in range(nlo)] + \
                                [hi_base + t for t in range(nhi)]
                        lo_base += nlo
                        hi_base += nhi
                        ntot = nlo + nhi
                        aggT = psA.tile([P, P], FP32, space="PSUM", name="aggT", tag="aggT")
                        for k, gt in enumerate(tlist):
                            tid = gd["tid0"] + gt
                            pm = ppool.tile([P, P], FP16, name="pm", tag="pm")
                            nc.vector.tensor_scalar(
                                out=pm[:],
                                in0=iota_ps[:],
                                scalar1=colw_t[:, 2 * tid:2 * tid + 1],
                                scalar2=colw_t[:, 2 * tid + 1:2 * tid + 2],
                                op0=mybir.AluOpType.is_equal,
                                op1=mybir.AluOpType.mult,
                            )
                            nc.tensor.matmul(
                                out=aggT[:],
                                lhsT=gbuf[:, gt * P:(gt + 1) * P],
                                rhs=pm[:],
                                start=(k == 0),
                                stop=(k == ntot - 1),
                            )
                        aggT_sb = epool.tile([P, P], FP16, name="evict", tag="evict")
                        nc.scalar.copy(out=aggT_sb[:], in_=aggT[:])
                        h_ps = psB.tile([P, P], FP32, space="PSUM", name="hps", tag="hps")
                        nc.tensor.matmul(
                            out=h_ps[:], lhsT=aggT_sb[:], rhs=w_t[l][:],
                            start=True, stop=False,
                        )
                        nc.tensor.matmul(
                            out=h_ps[:], lhsT=ones_t[0:1, :], rhs=b_t[l][0:1, :],
                            start=False, stop=True,
                        )
                        h_sb = hpool.tile([P, P], out_dt, name="hout", tag=f"hout{l}")
                        nc.scalar.activation(out=h_sb[:], in_=h_ps[:], func=relu)
                        nc.sync.dma_start(
                            out=dst_ap[s * P:(s + 1) * P, :], in_=h_sb[:]
                        )

            layer(0, x16_full[0:LO_SPLIT, :], x16_full[LO_SPLIT:NFULL, :],
                  h_shard[:], FP16)

            nc.gpsimd.collective_compute(
                "AllGather", mybir.AluOpType.bypass,
                replica_groups=[list(range(NC))],
                ins=[h_shard[:]], outs=[h_full[:]],
            )

            layer(1, h_full[0:LO_SPLIT, :], h_full[LO_SPLIT:NFULL, :],
                  out[:], FP32)

    nc.finalize()
    return nc


def kernel(x, edge_index, edge_weight, W1, b1, W2, b2):
    global last_run_results
    x = np.ascontiguousarray(np.asarray(x, dtype=np.float32))
    edge_index = np.asarray(edge_index)
    edge_weight = np.asarray(edge_weight, dtype=np.float32)

    block_at, sched, idx_np, colw_np, counts_np = _prep(edge_index, edge_weight)
    n16 = idx_np[0].shape[1]
    nc = _build(sched, n16)

    iota_np = np.broadcast_to(np.arange(P, dtype=np.float32), (P, P)).copy()
    xpad = np.zeros((NFULL, NFEAT), np.float32)
    xpad[:N_NODES] = x
    in_maps = []
    for c in range(NC):
        xsh = np.concatenate(
            [xpad[b * P:(b + 1) * P] for b in block_at[c]], axis=0
        )
        in_maps.append({
            "xsh": np.ascontiguousarray(xsh),
            "W1": np.ascontiguousarray(W1, dtype=np.float32),
            "W2": np.ascontiguousarray(W2, dtype=np.float32),
            "b1": np.ascontiguousarray(b1, dtype=np.float32).reshape(1, NFEAT),
            "b2": np.ascontiguousarray(b2, dtype=np.float32).reshape(1, NFEAT),
            "iota": iota_np,
            "idx": idx_np[c],
            "colw": colw_np[c],
        })

    import os
    trace = bool(int(os.environ.get("GCN_TRACE", "0")))
    res = run_bass_kernel_spmd(nc, in_maps, list(range(NC)), trace=trace)
    last_run_results = res

    full = np.zeros((NFULL, NFEAT), np.float32)
    for c in range(NC):
        shard = res.results[c]["out"]
        for s in range(SLOTS):
            b = int(block_at[c, s])
            full[b * P:(b + 1) * P] = shard[s * P:(s + 1) * P]
    return full[:N_NODES]

